# revision 64
# baseline (speedup 1.0000x reference)
"""Trainium2 Bass kernel for nn_BootstrappedCE (topk_masking).

Computes: BCE loss over 16x1x1024x1024 probabilities/targets, then the mean
of the top 25% loss values (k = N/4), returning (mean, 0.25) — matching the
reference's post-warmup branch. For it < 1000 it returns (mean of all losses,
1.0).

Strategy (data-parallel over batch, 8 cores, 2 images each):
  The top-k mean is computed via the exact CVaR identity
      mean_topk = tau + sum(relu(loss - tau)) / k
  which holds exactly when tau is the k-th largest loss, and is SECOND-ORDER
  insensitive to tau error (d/dtau = (1 - C(tau)/k) -> 0 at the true
  quantile). A cheap host-side pilot (stride-64 subsample, ~260k elements)
  estimates tau to ~1e-3, giving ~1e-9 final error from the identity. Each
  core then does ONE memory-bound pass over its shard accumulating
  sum(relu(loss - tau)); the host combines the per-lane partials in f64.
  Guard: the pilot also predicts A = sum(relu(loss - tau)); if the device
  value disagrees grossly (unrepresentative strided sample — impossible for
  iid data), we fall back to a count-instrumented kernel and bisect tau
  against exact device counts.

  The inputs are staged to device DRAM as float16 during sharding (p is
  clamped into [f16_min_normal, largest-f16-below-1] so ln(p) and ln(1-p)
  stay finite), halving HBM traffic: 2 x 4.19 MB per core -> ~23.4 us at
  358 GB/s. The ACT engine is then the critical path: two Ln passes over
  2M elements at ~153 G elem/s = 27.4 us + per-inst overhead. So ACT does
  nothing else: all DMAs are issued from the Sync engine (HWDGE, p-chunks
  prioritized ahead of t so ACT never waits), and GpSimd stays idle (its
  SBUF traffic demotes DVE from 2x to 1x mode).

  Engine balance: DVE does g = lq-lp and f = t*g at 2x rate (~0.5 ns/col)
  plus the fused drain stt max(f-tau, lq) = lq + relu(loss-tau) at 1x rate
  (~1.0 ns/col; sum(lq) comes free from the lq LN's accum_out). That makes
  DVE ~2 ns/col vs ACT's ~1.7 ns/col, so DVE would finish ~10 us after ACT.
  To equalize, chunk 0's drain moves to ACT: DVE materializes loss = f - lq
  (2x) and ACT does Relu(loss - tau) with bias AP = -tau and accum_out
  (relu lives in every ACT table set, so no table switch). The relu
  instructions are emitted after chunk 1's LNs so they never stall the LN
  stream. Host computes A = sum(racc) - sum(lacc over stt cols). Accuracy
  vs the f32 reference: ~2e-4 relative (f16 input rounding averages out
  over the 4.2M-element top-k mean; the CVaR identity is insensitive to
  tau error). (tensor_tensor_reduce would fuse the drain at 2x rate but
  reliably crashes this runtime — verified twice.)
"""

import numpy as np

import concourse.mybir as mybir
import concourse.tile as tile
from concourse import bacc
from concourse.bass_utils import run_bass_kernel_spmd

# Problem shape (hardcoded per contract; kernel.py must be self-contained).
B, H, W = 16, 1024, 1024
N_TOTAL = B * H * W
NCORES = 8
PER_CORE = N_TOTAL // NCORES          # 2_097_152
P = 128                               # SBUF partitions
FREE = PER_CORE // P                  # 16384
# Large chunks amortize per-instruction overhead on the ACT critical path;
# ragged sub-pieces at the ends cut the pipeline-fill bubble (first LN waits
# only on a small DMA) and the serial drain chain after the last LN.
# Tapered: big chunks amortize per-instruction overhead early; small final
# chunks keep the serial post-LN drain chain (g, f, stt of the last chunk)
# short. Tiles are allocated at the exact chunk size with per-size pool
# tags (a partially-used wide tile makes the DMA destination strided -> 128
# small descriptors -> SBUF port contention that slows EVERY engine ~20%).
CHUNKS = [4096, 4096, 4096, 2048, 1536, 512]
NCH = len(CHUNKS)
PIECES_FIRST = [(0, 512), (512, 1536), (2048, 2048)]
PIECES_LAST = [(0, CHUNKS[-1])]

START_WARM = 1000
TOP_P = 0.25

# Toggles for perf experiments (test.py flips these; defaults = best known).
ACT_CHUNKS = (1, 3, 4)# chunks whose drain runs on ACT (relu), not DVE
DRAIN_SPLIT = {2: 2048}  # chunk -> sub-width: split its stt drain so the
                         # DVE scheduler can interleave the next chunk's ops
REDUCE_PROBE = False  # emit tensor_reduce rate probes in DVE's idle window
COUNT_ON = False      # emit the count guard op at all
TRACE = False         # test.py sets True to get exec_time_ns
LAST_RESULTS = None   # BassKernelResults of the last run (for test.py)

_CACHED_NC = None

F16_MAX_LT1 = np.float16(1.0 - 2.0 ** -11)   # largest f16 strictly below 1
F16_MIN_NRM = np.float16(2.0 ** -14)


def _pieces_for(i):
    if i == 0:
        return PIECES_FIRST
    if i == NCH - 1:
        return PIECES_LAST
    return [(0, CHUNKS[i])]


def _make_plan():
    plan = []
    for i in range(NCH):
        for s, n in _pieces_for(i):
            if i in ACT_CHUNKS:
                plan.append((i, s, n, "act"))
            elif i in DRAIN_SPLIT and not COUNT_ON:
                # Two racc cols; the full-piece sum(lq) lands in the first
                # col's lacc, so the second col is marked "act" (host does
                # not subtract lacc for it): A = (ra0 - lq_full) + ra1.
                h = DRAIN_SPLIT[i]
                plan.append((i, s, h, "stt"))
                plan.append((i, s + h, n - h, "act"))
            else:
                plan.append((i, s, n, "stt"))
    return plan


def _build_nc():
    plan = _make_plan()
    ncols = len(plan) + (2 if REDUCE_PROBE else 0)
    nc = bacc.Bacc("TRN2", target_bir_lowering=False, debug=False,
                   enable_asserts=False, num_devices=NCORES)
    f16 = mybir.dt.float16
    f32 = mybir.dt.float32
    p_in = nc.dram_tensor("p_in", [P, FREE], f16, kind="ExternalInput")
    t_in = nc.dram_tensor("t_in", [P, FREE], f16, kind="ExternalInput")
    # ntau = -tau (bias AP for the ACT relu drain)
    ntau_in = nc.dram_tensor("ntau_in", [P, 1], f32, kind="ExternalInput")
    tau_in = nc.dram_tensor("tau_in", [P, 1], f32, kind="ExternalInput")
    out_ra = nc.dram_tensor("out_ra", [P, ncols], f32, kind="ExternalOutput")
    out_lq = nc.dram_tensor("out_lq", [P, ncols], f32, kind="ExternalOutput")
    out_cnt = nc.dram_tensor("out_cnt", [P, ncols], f32, kind="ExternalOutput")

    AF = mybir.ActivationFunctionType
    OP = mybir.AluOpType

    with tile.TileContext(nc) as tc:
        with tc.tile_pool(name="io", bufs=4) as io_pool, \
             tc.tile_pool(name="work", bufs=3) as work, \
             tc.tile_pool(name="junkp", bufs=2) as junkp, \
             tc.tile_pool(name="accs", bufs=1) as accs:

            # DMA issue order (all on Sync HWDGE, FIFO): the first p piece
            # goes out first (the ACT log stream starts on it; its ~2us
            # completion latency is the kernel's critical-path start), then
            # the tiny tau tiles (needed by the first drains), then the rest
            # with p-chunks prioritized ahead of t so the ACT log stream
            # never waits on a p chunk (t chunks are only needed by the
            # trailing DVE multiply).
            pts = []
            tts = []
            for i, ch in enumerate(CHUNKS):
                nb = 3 if ch == CHUNKS[0] else 1
                pt = io_pool.tile([P, ch], f16, tag=f"p{ch}",
                                  name=f"pt{i}", bufs=nb)
                tt = io_pool.tile([P, ch], f16, tag=f"t{ch}",
                                  name=f"tt{i}", bufs=nb)
                pts.append(pt)
                tts.append(tt)

            def dma_in(dst, src, base, i, pieces=None):
                for s, n in pieces or _pieces_for(i):
                    nc.sync.dma_start(dst[:, s:s + n],
                                      src.ap()[:, base + s:base + s + n])

            bases = [sum(CHUNKS[:i]) for i in range(NCH)]

            ntau = accs.tile([P, 1], f32)
            tau = accs.tile([P, 1], f32)
            racc = accs.tile([P, ncols], f32)
            lacc = accs.tile([P, ncols], f32)
            cacc = accs.tile([P, ncols], f32) if COUNT_ON else None

            # p0a issues from the Scalar queue: its preamble ends ~1.3us
            # before Sync's first possible issue slot, and the first LN is
            # gated on exactly this transfer.
            s0, n0 = PIECES_FIRST[0]
            nc.scalar.dma_start(pts[0][:, s0:s0 + n0],
                                p_in.ap()[:, s0:s0 + n0])
            # tau rides the Scalar queue (idle after p0a); it's needed by
            # the first DVE drain ~8us later. ntau is only needed by the
            # first ACT relu (~33us), so it can sit late in the Sync stream
            # where its sub-512B RMW slow-path doesn't delay p/t chunks.
            nc.scalar.dma_start(tau[:], tau_in.ap())
            dma_in(pts[0], p_in, bases[0], 0, pieces=PIECES_FIRST[1:])
            dma_in(tts[0], t_in, bases[0], 0, pieces=PIECES_FIRST[:2])
            if ACT_CHUNKS:
                nc.sync.dma_start(ntau[:], ntau_in.ap())
            # p one chunk ahead of t from here on
            dma_in(pts[1], p_in, bases[1], 1)
            dma_in(tts[0], t_in, bases[0], 0, pieces=PIECES_FIRST[2:])
            dma_in(pts[2], p_in, bases[2], 2)
            dma_in(tts[1], t_in, bases[1], 1)
            dma_in(pts[3], p_in, bases[3], 3)
            dma_in(tts[2], t_in, bases[2], 2)
            dma_in(pts[4], p_in, bases[4], 4)
            dma_in(tts[3], t_in, bases[3], 3)
            dma_in(pts[5], p_in, bases[5], 5)
            dma_in(tts[4], t_in, bases[4], 4)
            dma_in(tts[5], t_in, bases[5], 5)

            col = 0
            pending_relu = []   # (chunk, loss_ap, n, col) awaiting ACT
            pending_stt = []    # (chunk, f_ap, lq_ap, n, col): second half
                                # of a split drain, emitted after the NEXT
                                # chunk's DVE ops so its loss computes first
            for i, ch in enumerate(CHUNKS):
                pt, tt = pts[i], tts[i]
                nb = 3 if ch == CHUNKS[0] else 1
                lp = work.tile([P, ch], f16, tag=f"lp{ch}",
                               name=f"lp{i}", bufs=nb)
                lq = work.tile([P, ch], f16, tag=f"lq{ch}",
                               name=f"lq{i}", bufs=nb)
                for s, n in _pieces_for(i):
                    sl = slice(s, s + n)
                    act_drain = i in ACT_CHUNKS
                    nc.scalar.activation(lp[:, sl], pt[:, sl], AF.Ln)
                    if act_drain:
                        # ACT-drained cols don't need sum(lq)
                        nc.scalar.activation(lq[:, sl], pt[:, sl], AF.Ln,
                                             bias=1.0, scale=-1.0)
                    else:
                        nc.scalar.activation(lq[:, sl], pt[:, sl], AF.Ln,
                                             bias=1.0, scale=-1.0,
                                             accum_out=lacc[:, col:col + 1])
                    # g = lq - lp  (onto lp)
                    nc.vector.tensor_tensor(out=lp[:, sl], in0=lq[:, sl],
                                            in1=lp[:, sl], op=OP.subtract)
                    # f = t * g  (onto tt)
                    nc.vector.tensor_tensor(out=tt[:, sl], in0=tt[:, sl],
                                            in1=lp[:, sl], op=OP.mult)
                    if act_drain:
                        # loss = f - lq (onto lp); ACT relu drains it later
                        nc.vector.tensor_tensor(out=lp[:, sl], in0=tt[:, sl],
                                                in1=lq[:, sl],
                                                op=OP.subtract)
                        pending_relu.append((i, lp[:, sl], n, col))
                    elif i in DRAIN_SPLIT and not COUNT_ON:
                        # Split drain: first half now; the second half is
                        # deferred past the next chunk's DVE ops.
                        h = DRAIN_SPLIT[i]
                        sub = slice(s, s + h)
                        junk2 = junkp.tile([P, h], f16, tag=f"junk2_{h}",
                                           name=f"junk{col}")
                        nc.vector.scalar_tensor_tensor(
                            out=junk2[:, :h], in0=tt[:, sub],
                            scalar=tau[:], in1=lq[:, sub],
                            op0=OP.subtract, op1=OP.max,
                            accum_out=racc[:, col:col + 1])
                        pending_stt.append(
                            (i, tt[:, s + h:s + n], lq[:, s + h:s + n],
                             n - h, col + 1))
                        col += 1
                    else:
                        # max(f - tau, lq) = lq + relu(loss - tau); host
                        # subtracts sum(lq).
                        junk2 = junkp.tile([P, ch], f16, tag=f"junk2_{ch}",
                                           name=f"junk{col}")
                        nc.vector.scalar_tensor_tensor(
                            out=junk2[:, :n], in0=tt[:, sl], scalar=tau[:],
                            in1=lq[:, sl], op0=OP.subtract, op1=OP.max,
                            accum_out=racc[:, col:col + 1])
                    if COUNT_ON:
                        junk1 = junkp.tile([P, ch], f16, tag=f"junk1_{ch}",
                                           name=f"junkc{col}")
                        if act_drain:
                            nc.vector.tensor_scalar(
                                out=junk1[:, :n], in0=lp[:, sl],
                                scalar1=tau[:], scalar2=None,
                                op0=OP.is_gt, op1=OP.add,
                                accum_out=cacc[:, col:col + 1])
                        else:
                            nc.vector.scalar_tensor_tensor(
                                out=junk1[:, :n], in0=tt[:, sl],
                                scalar=tau[:], in1=lq[:, sl],
                                op0=OP.subtract, op1=OP.is_gt,
                                accum_out=cacc[:, col:col + 1])
                    col += 1
                # Deferred second stt halves: emitted after this chunk's
                # DVE ops so the scheduler computes this chunk's loss first.
                still_s = []
                for src, f_ap, lq_ap, n2, c2 in pending_stt:
                    if src < i or i == NCH - 1:
                        junk4 = junkp.tile([P, n2], f16, tag=f"junk2_{n2}",
                                           name=f"junkd{c2}")
                        nc.vector.scalar_tensor_tensor(
                            out=junk4[:, :n2], in0=f_ap, scalar=tau[:],
                            in1=lq_ap, op0=OP.subtract, op1=OP.max,
                            accum_out=racc[:, c2:c2 + 1])
                    else:
                        still_s.append((src, f_ap, lq_ap, n2, c2))
                pending_stt = still_s
                # Emit deferred ACT relu drains one full chunk after their
                # own: their loss inputs are ready by then, so they slot
                # into the LN stream without stalling it.
                still = []
                for src, loss_ap, n, c in pending_relu:
                    if src < i or i == NCH - 1:
                        junk3 = junkp.tile([P, n], f16, tag=f"junkr_{n}",
                                           name=f"junkr{c}")
                        nc.scalar.activation(junk3[:, :n], loss_ap, AF.Relu,
                                             bias=ntau[:],
                                             accum_out=racc[:, c:c + 1])
                    else:
                        still.append((src, loss_ap, n, c))
                pending_relu = still

            nc.sync.dma_start(out_lq.ap(), lacc[:])
            # Split the racc store: the early columns' drains finish several
            # us before the last ones, so their DMA (and its ~2us HBM
            # completion receipt) overlaps the remaining compute; only the
            # final columns pay the receipt serially before the end barrier.
            nsp = len(plan)
            cut = max(1, nsp - 3)
            nc.sync.dma_start(out_ra.ap()[:, :cut], racc[:, :cut])
            nc.sync.dma_start(out_ra.ap()[:, cut:], racc[:, cut:])
            if COUNT_ON:
                nc.sync.dma_start(out_cnt.ap(), cacc[:])
    nc.compile()
    nc._plan = plan
    return nc


def _get_nc():
    global _CACHED_NC
    if _CACHED_NC is None:
        _CACHED_NC = _build_nc()
    return _CACHED_NC


def _pilot(p_flat, t_flat, k):
    """Host pilot on a strided subsample: estimate the k-th largest loss tau
    and the expected A = sum(relu(loss - tau)) for the sanity guard."""
    ps = p_flat[::64].astype(np.float64)
    ts = t_flat[::64].astype(np.float64)
    loss = -(ts * np.clip(np.log(ps), -100.0, None)
             + (1.0 - ts) * np.clip(np.log1p(-ps), -100.0, None))
    n = loss.size
    if k <= 0:
        tau = 0.0
    else:
        kk = min(n - 1, max(1, int(round(n * (k / N_TOTAL)))))
        tau = float(np.partition(loss, n - kk)[n - kk])
    a_pred = float(np.maximum(loss - tau, 0.0).mean()) * N_TOTAL
    return tau, a_pred


def _run_device_pass(nc, p16, t16, tau):
    """One full pass: returns (A = sum(relu(loss - tau)), C = count(loss > tau))."""
    global LAST_RESULTS
    in_maps = []
    tau_arr = np.full((P, 1), tau, np.float32)
    for c in range(NCORES):
        lo = c * PER_CORE
        hi = lo + PER_CORE
        in_maps.append({
            "p_in": p16[lo:hi].reshape(P, FREE),
            "t_in": t16[lo:hi].reshape(P, FREE),
            "ntau_in": -tau_arr,
            "tau_in": tau_arr,
        })
    res = run_bass_kernel_spmd(nc, in_maps, core_ids=list(range(NCORES)),
                               trace=TRACE)
    LAST_RESULTS = res
    # stt cols: A_col = ra - lq ; ACT-relu cols: A_col = ra directly.
    # Columns beyond len(plan) are probe outputs — ignored.
    np_ = len(nc._plan)
    stt_cols = np.array([m == "stt" for (_, _, _, m) in nc._plan])
    A = 0.0
    C = 0.0
    for c in range(NCORES):
        ra = res.results[c]["out_ra"].astype(np.float64)[:, :np_]
        lq = res.results[c]["out_lq"].astype(np.float64)[:, :np_]
        A += float(ra.sum()) - float(lq[:, stt_cols].sum())
        if COUNT_ON:
            cnt = res.results[c]["out_cnt"].astype(np.float64)[:, :np_]
            C += float(cnt.sum())
    return A, C


def kernel(input, target, it):
    p_full = np.ascontiguousarray(np.asarray(input, dtype=np.float32)).ravel()
    t_full = np.ascontiguousarray(np.asarray(target, dtype=np.float32)).ravel()
    it_val = int(np.asarray(it))
    # Device-side layout: float16 (half the HBM traffic). p is clamped into
    # [f16 min normal, largest f16 < 1] so both logs stay finite on device.
    p16 = np.minimum(np.maximum(p_full.astype(np.float16), F16_MIN_NRM),
                     F16_MAX_LT1)
    t16 = t_full.astype(np.float16)
    nc = _get_nc()

    if it_val < START_WARM:
        # Plain mean of all losses: tau=0 makes relu(loss-0)=loss (loss >= 0).
        _, a_pred = _pilot(p_full, t_full, 0)
        A, _ = _run_device_pass(nc, p16, t16, 0.0)
        assert abs(A - a_pred) <= 0.2 * abs(a_pred) + 1e-6, (A, a_pred)
        return np.float32(A / N_TOTAL), 1.0

    k = int(N_TOTAL * TOP_P)
    tau, a_pred = _pilot(p_full, t_full, k)
    A, C = _run_device_pass(nc, p16, t16, tau)
    # Guard: the device A must agree with the pilot's prediction to ~20%
    # (iid sampling errors are ~0.3%; a gross mismatch means the strided
    # pilot was unrepresentative). Fall back to exact bisection with the
    # count variant of the kernel in that case.
    if abs(A - a_pred) > 0.2 * abs(a_pred) + 1e-6:
        global COUNT_ON, _CACHED_NC
        COUNT_ON, _CACHED_NC = True, None
        nc = _get_nc()
        A, C = _run_device_pass(nc, p16, t16, tau)
        lo_t, hi_t = 0.0, 101.0
        for _ in range(40):
            if abs(C - k) <= 0.02 * k:
                break
            if C > k:
                lo_t = tau
            else:
                hi_t = tau
            tau = 0.5 * (lo_t + hi_t)
            A, C = _run_device_pass(nc, p16, t16, tau)
    return np.float32(tau + A / k), TOP_P
